# revision 48
# baseline (speedup 1.0000x reference)
"""Trainium2 Bass kernel for nn_LocalAggregator (GNN message passing).

Computation (reference semantics):
    te    = p0*exp(-t) + p1
    h     = [hidden[..., :127] | te]
    e_k   = leaky_relu((h*a_k) @ h^T, 0.2)          k = 0..3
    alpha = softmax(select_by_adj(e_k, adj, -inf))   over last axis
    out   = alpha @ h

Device strategy (pure data-parallel over batch, 8 cores x 8 batches):
  - Score planes e_k are symmetric bilinear forms, so we compute them in
    [j, i] layout (neighbor j on partitions).  The softmax denominator
    (sum over j) then falls out of the aggregation matmul as an extra
    ones-column of h -- no transposes, no reductions.
  - The 4-way adjacency select runs as a copy_predicated ladder over the
    PSUM score planes using host-shipped uint8 bit-plane masks
    (b0 = lsb(adj-1), b1 = adj>=3).  That is the only DVE work.  (A
    fused free-1024 pair-select was tried: it saves 126ns/jc of DVE but
    coarsens PSUM recycling to 2-bank granularity, which costs more in
    pipeline depth than it saves.)
  - The adj==0 kill is a POST-exp multiply by a host-shipped bf16 {0,1}
    plane on the Pool engine: n = exp(prelu(e_sel)) * mz.  Post-exp, the
    Pool hop sits outside the ACT prelu->exp chain, so its latency is
    absorbed by the aggregation tail (which overlaps the next batch's
    score phase on the PE).  Pre-exp Pool placement was tried and cost
    13us: the in-order ACT queue couples exp(jc3)'s wait into the next
    batch's prelu.  (Pool cannot touch PSUM; only add/sub/mul pass the
    walrus ISA check for TensorTensor on Pool.)
  - Prelu (leaky relu) and Exp run on the scalar engine.
  - The aggregation leaves the device UNNORMALIZED together with its Z
    (ones) column as fp16; the 1/Z divide happens on the host.  This
    keeps reciprocal/scale instructions off the in-order engine queues.
  - Emission order is software-pipelined at jc-pair granularity: batch
    b-1's two aggregation pairs are emitted between batch b's jc-pairs,
    so aggregation pair 1 (gated on pair 0's ACT copy) never blocks
    batch b+1's score matmuls on the in-order PE queue.
  - All PSUM (score planes and paired aggregation accumulators) comes
    from one unified 8x1-bank pool; a few tiny warm-up matmuls absorb
    the cold-start PE costs during the input DMA.  The last batch's
    final-jc funnel is half-chunked (256-wide prelu/exp/kill on the
    idle DVE) so the drain-tail aggregation starts earlier.  Steady
    state is 100% DVE-saturated; the ~5us ramp and ~4us tail that
    remain are DMA/semaphore framework floors (625ns/DMA HWDGE
    pipeline, ~1.3us DGE issue latency, start/end barriers).
"""

import os
import sys

import numpy as np

for _p in ("/opt/trn_rl_repo", "/root/.axon_site/_ro/trn_rl_repo"):
    if os.path.isdir(_p) and _p not in sys.path:
        sys.path.insert(0, _p)

B, N, DIM = 64, 512, 128
NCORES = 8
BPC = B // NCORES          # batches per core
JC = N // 128              # j-chunks per batch
IC = N // 128              # i-chunks per batch
HAUG = 132                 # 128 dims + ones col + pad
LEAKY_ALPHA = 0.2

# packed bf16 blob offsets (per-partition free-dim layout)
OFF_HT = 0                     # hT            [128, N]      (blob A)
OFF_HTK = N                    # hTk (4x)      [128, 4, N]   (blob A)
BINAW = 5 * N
OFF_HAUG = 0                   # haug          [128, JC, HAUG]  (blob B)
OFF_MZ = JC * HAUG             # mz (adj>0)    [128, JC, N]     (blob B)
BINBW = JC * HAUG + JC * N
MMW = 2 * JC * N               # u8 blob: b0m then b1m

_CACHE = {}


def _build_nc(repeat=1):
    import concourse.bass as bass
    from concourse import bacc, mybir
    from concourse.tile import TileContext

    bf16 = mybir.dt.bfloat16
    f16 = mybir.dt.float16
    f32 = mybir.dt.float32
    u8 = mybir.dt.uint8
    act = mybir.ActivationFunctionType

    nc = bacc.Bacc(None, target_bir_lowering=False)

    bina_d = nc.declare_dram_parameter("bina", [BPC, 128, BINAW], bf16, isOutput=False)
    mm_d = nc.declare_dram_parameter("mm", [BPC, 128, MMW], u8, isOutput=False)
    binb_d = nc.declare_dram_parameter("binb", [BPC, 128, BINBW], bf16, isOutput=False)
    out_d = nc.declare_dram_parameter("out", [BPC, 128, IC, DIM + 1], f16, isOutput=True)

    with TileContext(nc) as tc:
        with (
            tc.tile_pool(name="inp", bufs=7) as inp,
            tc.tile_pool(name="work", bufs=32) as work,
            tc.tile_pool(name="outp", bufs=4) as outp,
            tc.tile_pool(name="npool", bufs=32) as npool,
            tc.tile_pool(name="pse", bufs=8, space=bass.MemorySpace.PSUM) as pse,
        ):
            warm = work.tile([128, 128], bf16, tag="warm", name="warm")
            nc.vector.memset(warm[:], 0.0)
            wps = pse.tile([128, N], f32, tag="e", name="warmps")
            for _ in range(4):
                nc.tensor.matmul(wps[:, 0:16], warm[:],
                                 warm[:, 0:16], start=True, stop=True)

            def score_dma(rep, b, first=False):
                bina1_t = inp.tile([128, 3 * N], bf16)
                mm_t = inp.tile([128, MMW], u8)
                bina2_t = inp.tile([128, 2 * N], bf16)
                binb_t = inp.tile([128, BINBW], bf16)
                if first:
                    # fine-grained ramp: land the jc0 ladder inputs ASAP
                    nc.sync.dma_start(out=bina1_t[:, 0:2 * N], in_=bina_d[b, :, 0:2 * N])
                    nc.sync.dma_start(out=bina1_t[:, 2 * N:], in_=bina_d[b, :, 2 * N:3 * N])
                    nc.sync.dma_start(out=mm_t[:, 0:JC * N], in_=mm_d[b, :, 0:JC * N])
                    nc.sync.dma_start(out=bina2_t[:], in_=bina_d[b, :, 3 * N:])
                    nc.sync.dma_start(out=mm_t[:, JC * N:], in_=mm_d[b, :, JC * N:])
                    nc.sync.dma_start(out=binb_t[:], in_=binb_d[b])
                else:
                    nc.sync.dma_start(out=bina1_t[:], in_=bina_d[b, :, 0:3 * N])
                    nc.sync.dma_start(out=bina2_t[:], in_=bina_d[b, :, 3 * N:])
                    nc.sync.dma_start(out=mm_t[:], in_=mm_d[b])
                    nc.sync.dma_start(out=binb_t[:], in_=binb_d[b])
                return bina1_t, bina2_t, mm_t, binb_t

            def score_jcs(tiles, rep, b, ns, jcs, last=False):
                bina1_t, bina2_t, mm_t, binb_t = tiles
                hT = bina1_t[:, OFF_HT:OFF_HT + N]
                for jc in jcs:
                    e = [pse.tile([128, N], f32, tag="e", name=f"e{rep}_{b}_{jc}_{k}")
                         for k in range(4)]
                    for k in range(4):
                        # e_k[j, i] = sum_d hT[d, j-chunk] * (a_k . h)^T[d, i]
                        nc.tensor.matmul(
                            e[k][:],
                            hT[:, jc * 128:(jc + 1) * 128],
                            (bina1_t[:, (1 + k) * N:(2 + k) * N] if k < 2
                             else bina2_t[:, (k - 2) * N:(k - 1) * N]),
                            start=True,
                            stop=True,
                        )

                    b0m = mm_t[:, jc * N:(jc + 1) * N]
                    b1m = mm_t[:, (JC + jc) * N:(JC + jc + 1) * N]

                    # 4-way select ladder -> e[0] holds e_{adj-1}
                    nc.vector.copy_predicated(e[0][:], b0m, e[1][:])
                    nc.vector.copy_predicated(e[2][:], b0m, e[3][:])
                    nc.vector.copy_predicated(e[0][:], b1m, e[2][:])

                    npre = work.tile([128, N], bf16)
                    nexp = work.tile([128, N], bf16)
                    n = npool.tile([128, N], bf16, tag="n", name=f"n{rep}_{b}_{jc}")
                    if last and jc == JC - 1:
                        # drain tail: chunk the final funnel in halves (the
                        # aggregation consumes n in 128-wide slices) with the
                        # kill on the idle DVE, so the tail aggs start earlier
                        for hf in range(2):
                            sl = slice(hf * 256, (hf + 1) * 256)
                            nc.scalar.activation(
                                npre[:, sl], e[0][:, sl], act.Prelu,
                                alpha=LEAKY_ALPHA
                            )
                            nc.scalar.activation(nexp[:, sl], npre[:, sl], act.Exp)
                            nc.vector.tensor_mul(
                                n[:, sl],
                                binb_t[:, OFF_MZ + jc * N + hf * 256:
                                       OFF_MZ + jc * N + (hf + 1) * 256],
                                nexp[:, sl],
                            )
                    else:
                        nc.scalar.activation(
                            npre[:], e[0][:], act.Prelu, alpha=LEAKY_ALPHA
                        )
                        nc.scalar.activation(nexp[:], npre[:], act.Exp)
                        # n = nexp * mz  (adj==0 -> n == 0), on Pool, off the
                        # ACT critical chain
                        nc.gpsimd.tensor_mul(
                            n[:],
                            binb_t[:, OFF_MZ + jc * N:OFF_MZ + (jc + 1) * N],
                            nexp[:],
                        )
                    ns.append(n)

            def agg_pair(st, icp, last=False):

                rep, b, ns, binb_t, outt = st
                # two aggs packed in one PSUM bank (528B each); the last
                # batch puts its second pair in a (drained) score bank so
                # both pairs accumulate concurrently
                agg = pse.tile([128, N], f32, tag="e",
                               name=f"agg{rep}_{b}_{icp}")[:, 0:2 * HAUG] \
                    .rearrange("p (two h) -> p two h", two=2)
                for half in range(2):
                    ic = 2 * icp + half
                    for jc in range(JC):
                        # out_un[i-chunk, 0:129] += n[:, i-chunk].T @ [h | 1]
                        nc.tensor.matmul(
                            agg[:, half, 0:DIM + 1],
                            ns[jc][:, ic * 128:(ic + 1) * 128],
                            binb_t[:, OFF_HAUG + jc * HAUG:OFF_HAUG + jc * HAUG + DIM + 1],
                            start=(jc == 0),
                            stop=(jc == JC - 1),
                        )
                # ship agg + Z column unnormalized; host divides by Z.
                # The last batch's second pair copies on the (idle) DVE so
                # both copies run concurrently in the drain tail.
                if last and icp == 1:
                    nc.vector.tensor_copy(
                        outt[:, 2 * icp:2 * icp + 2, 0:DIM + 1],
                        agg[:, :, 0:DIM + 1],
                    )
                else:
                    nc.scalar.activation(
                        outt[:, 2 * icp:2 * icp + 2, 0:DIM + 1],
                        agg[:, :, 0:DIM + 1], act.Copy
                    )
                nc.sync.dma_start(
                    out=out_d[b, :, 2 * icp:2 * icp + 2],
                    in_=outt[:, 2 * icp:2 * icp + 2],
                )

            # software pipeline: batch b-1's two aggregation pairs are
            # emitted between batch b's jc-pairs, so agg pair 1 (which waits
            # on pair 0's ACT copy) never blocks batch b+1's score matmuls
            # on the in-order PE queue
            pend = None
            seq = [(r, bb) for r in range(repeat) for bb in range(BPC)]
            for i, (rep, b) in enumerate(seq):
                last = i == len(seq) - 1
                tiles = score_dma(rep, b, first=(i == 0))
                outt = outp.tile([128, IC, DIM + 1], f16, tag="outt",
                                 name=f"outt{rep}_{b}")
                ns = []
                score_jcs(tiles, rep, b, ns, [0, 1, 2, 3], last=last)
                if pend is not None:
                    agg_pair(pend, 0)
                    agg_pair(pend, 1)
                pend = (rep, b, ns, tiles[3], outt)
            agg_pair(pend, 0, last=True)
            agg_pair(pend, 1, last=True)

    nc.compile()
    return nc


def _get_nc():
    if "nc" not in _CACHE:
        _CACHE["nc"] = _build_nc()
    return _CACHE["nc"]


def _host_prep(hidden, adj, input_times, a0, a1, a2, a3, p0, p1):
    import ml_dtypes

    bf16 = ml_dtypes.bfloat16

    hidden = np.asarray(hidden, dtype=np.float32)
    adj = np.asarray(adj)
    input_times = np.asarray(input_times, dtype=np.float32)

    te = np.asarray(p0, np.float32) * np.exp(-input_times) + np.asarray(p1, np.float32)
    h = np.concatenate([hidden[:, :, :-1], te[:, :, None]], axis=2)      # [B,N,128] f32

    hT = np.swapaxes(h, 1, 2)                                            # [B,128,N]
    A = np.stack([a0, a1, a2, a3], 0).astype(np.float32)                 # [4,128]

    bina = np.zeros((B, 128, BINAW), bf16)
    bina[:, :, OFF_HT:OFF_HT + N] = hT.astype(bf16)
    for k in range(4):
        bina[:, :, OFF_HTK + k * N:OFF_HTK + (k + 1) * N] = \
            (A[k][None, :, None] * hT).astype(bf16)

    # haug[b, jp, jc, c] = h[b, jc*128+jp, c] (+ ones col)
    binb = np.zeros((B, 128, BINBW), bf16)
    haug = np.zeros((B, N, HAUG), np.float32)
    haug[:, :, :DIM] = h
    haug[:, :, DIM] = 1.0
    haug = haug.reshape(B, JC, 128, HAUG).transpose(0, 2, 1, 3)
    binb[:, :, OFF_HAUG:OFF_HAUG + JC * HAUG] = \
        haug.reshape(B, 128, JC * HAUG).astype(bf16)

    def chunkT(m):
        # mask[b, i, j] -> transposed + chunked [b, jp, jc*N + i]
        mT = np.swapaxes(m, 1, 2)
        return mT.reshape(B, JC, 128, N).transpose(0, 2, 1, 3).reshape(B, 128, JC * N)

    mz = (adj != 0).astype(np.float32)
    binb[:, :, OFF_MZ:] = chunkT(mz).astype(bf16)

    mmb = np.zeros((B, 128, MMW), np.uint8)
    b0 = (((adj - 1) & 1) * (adj > 0)).astype(np.uint8)
    b1 = (adj >= 3).astype(np.uint8)
    mmb[:, :, :JC * N] = chunkT(b0)
    mmb[:, :, JC * N:] = chunkT(b1)

    in_maps = []
    for c in range(NCORES):
        s = slice(c * BPC, (c + 1) * BPC)
        in_maps.append({"bina": bina[s], "binb": binb[s], "mm": mmb[s]})
    return in_maps


def run(inputs, trace=False, **spmd_kwargs):
    """Full pipeline; returns (output, BassKernelResults)."""
    from concourse import bass_utils

    in_maps = _host_prep(**inputs)
    nc = _get_nc()
    res = bass_utils.run_bass_kernel_spmd(
        nc, in_maps, core_ids=list(range(NCORES)), trace=trace, **spmd_kwargs
    )
    outs = []
    for r in res.results:
        o = np.asarray(r["out"], np.float32)          # [BPC, 128, IC, DIM+1]
        o = o[:, :, :, 0:DIM] / o[:, :, :, DIM:DIM + 1]
        outs.append(o.transpose(0, 2, 1, 3).reshape(BPC, N, DIM))
    full = np.concatenate(outs, axis=0)
    return full, res


def kernel(**inputs) -> np.ndarray:
    out, _ = run(inputs, trace=False)
    return out


# revision 49
# speedup vs baseline: 1.0031x; 1.0031x over previous
"""Trainium2 Bass kernel for nn_LocalAggregator (GNN message passing).

Computation (reference semantics):
    te    = p0*exp(-t) + p1
    h     = [hidden[..., :127] | te]
    e_k   = leaky_relu((h*a_k) @ h^T, 0.2)          k = 0..3
    alpha = softmax(select_by_adj(e_k, adj, -inf))   over last axis
    out   = alpha @ h

Device strategy (pure data-parallel over batch, 8 cores x 8 batches):
  - Score planes e_k are symmetric bilinear forms, so we compute them in
    [j, i] layout (neighbor j on partitions).  The softmax denominator
    (sum over j) then falls out of the aggregation matmul as an extra
    ones-column of h -- no transposes, no reductions.
  - The 4-way adjacency select runs as a copy_predicated ladder over the
    PSUM score planes using host-shipped uint8 bit-plane masks
    (b0 = lsb(adj-1), b1 = adj>=3).  That is the only DVE work.  (A
    fused free-1024 pair-select was tried: it saves 126ns/jc of DVE but
    coarsens PSUM recycling to 2-bank granularity, which costs more in
    pipeline depth than it saves.)
  - The adj==0 kill is a POST-exp multiply by a host-shipped bf16 {0,1}
    plane on the Pool engine: n = exp(prelu(e_sel)) * mz.  Post-exp, the
    Pool hop sits outside the ACT prelu->exp chain, so its latency is
    absorbed by the aggregation tail (which overlaps the next batch's
    score phase on the PE).  Pre-exp Pool placement was tried and cost
    13us: the in-order ACT queue couples exp(jc3)'s wait into the next
    batch's prelu.  (Pool cannot touch PSUM; only add/sub/mul pass the
    walrus ISA check for TensorTensor on Pool.)
  - Prelu (leaky relu) and Exp run on the scalar engine.
  - The aggregation leaves the device UNNORMALIZED together with its Z
    (ones) column as fp16; the 1/Z divide happens on the host.  This
    keeps reciprocal/scale instructions off the in-order engine queues.
  - Emission order is software-pipelined: batch b-1's aggregation pair 0
    is emitted after batch b's jc2, pair 1 after jc3 (measured best of
    the interleave positions), so aggregation pair 1 (gated on pair 0's
    ACT copy) never blocks batch b+1's score matmuls on the in-order PE
    queue.  The last batch's second pair copies PSUM->SBUF on the idle
    DVE so both drain-tail copies run concurrently.
  - All PSUM (score planes and paired aggregation accumulators) comes
    from one unified 8x1-bank pool; a few tiny warm-up matmuls absorb
    the cold-start PE costs during the input DMA.  The last batch's
    final-jc funnel is half-chunked (256-wide prelu/exp/kill on the
    idle DVE) so the drain-tail aggregation starts earlier.  Steady
    state is 100% DVE-saturated; the ~5us ramp and ~4us tail that
    remain are DMA/semaphore framework floors (625ns/DMA HWDGE
    pipeline, ~1.3us DGE issue latency, start/end barriers).
"""

import os
import sys

import numpy as np

for _p in ("/opt/trn_rl_repo", "/root/.axon_site/_ro/trn_rl_repo"):
    if os.path.isdir(_p) and _p not in sys.path:
        sys.path.insert(0, _p)

B, N, DIM = 64, 512, 128
NCORES = 8
BPC = B // NCORES          # batches per core
JC = N // 128              # j-chunks per batch
IC = N // 128              # i-chunks per batch
HAUG = 132                 # 128 dims + ones col + pad
LEAKY_ALPHA = 0.2

# packed bf16 blob offsets (per-partition free-dim layout)
OFF_HT = 0                     # hT            [128, N]      (blob A)
OFF_HTK = N                    # hTk (4x)      [128, 4, N]   (blob A)
BINAW = 5 * N
OFF_HAUG = 0                   # haug          [128, JC, HAUG]  (blob B)
OFF_MZ = JC * HAUG             # mz (adj>0)    [128, JC, N]     (blob B)
BINBW = JC * HAUG + JC * N
MMW = 2 * JC * N               # u8 blob: b0m then b1m

_CACHE = {}


def _build_nc(repeat=1):
    import concourse.bass as bass
    from concourse import bacc, mybir
    from concourse.tile import TileContext

    bf16 = mybir.dt.bfloat16
    f16 = mybir.dt.float16
    f32 = mybir.dt.float32
    u8 = mybir.dt.uint8
    act = mybir.ActivationFunctionType

    nc = bacc.Bacc(None, target_bir_lowering=False)

    bina_d = nc.declare_dram_parameter("bina", [BPC, 128, BINAW], bf16, isOutput=False)
    mm_d = nc.declare_dram_parameter("mm", [BPC, 128, MMW], u8, isOutput=False)
    binb_d = nc.declare_dram_parameter("binb", [BPC, 128, BINBW], bf16, isOutput=False)
    out_d = nc.declare_dram_parameter("out", [BPC, 128, IC, DIM + 1], f16, isOutput=True)

    with TileContext(nc) as tc:
        with (
            tc.tile_pool(name="inp", bufs=7) as inp,
            tc.tile_pool(name="work", bufs=32) as work,
            tc.tile_pool(name="outp", bufs=4) as outp,
            tc.tile_pool(name="npool", bufs=32) as npool,
            tc.tile_pool(name="pse", bufs=8, space=bass.MemorySpace.PSUM) as pse,
        ):
            warm = work.tile([128, 128], bf16, tag="warm", name="warm")
            nc.vector.memset(warm[:], 0.0)
            wps = pse.tile([128, N], f32, tag="e", name="warmps")
            for _ in range(4):
                nc.tensor.matmul(wps[:, 0:16], warm[:],
                                 warm[:, 0:16], start=True, stop=True)

            def score_dma(rep, b, first=False):
                bina1_t = inp.tile([128, 3 * N], bf16)
                mm_t = inp.tile([128, MMW], u8)
                bina2_t = inp.tile([128, 2 * N], bf16)
                binb_t = inp.tile([128, BINBW], bf16)
                if first:
                    # fine-grained ramp: land the jc0 ladder inputs ASAP
                    nc.sync.dma_start(out=bina1_t[:, 0:2 * N], in_=bina_d[b, :, 0:2 * N])
                    nc.sync.dma_start(out=bina1_t[:, 2 * N:], in_=bina_d[b, :, 2 * N:3 * N])
                    nc.sync.dma_start(out=mm_t[:, 0:JC * N], in_=mm_d[b, :, 0:JC * N])
                    nc.sync.dma_start(out=bina2_t[:], in_=bina_d[b, :, 3 * N:])
                    nc.sync.dma_start(out=mm_t[:, JC * N:], in_=mm_d[b, :, JC * N:])
                    nc.sync.dma_start(out=binb_t[:], in_=binb_d[b])
                else:
                    nc.sync.dma_start(out=bina1_t[:], in_=bina_d[b, :, 0:3 * N])
                    nc.sync.dma_start(out=bina2_t[:], in_=bina_d[b, :, 3 * N:])
                    nc.sync.dma_start(out=mm_t[:], in_=mm_d[b])
                    nc.sync.dma_start(out=binb_t[:], in_=binb_d[b])
                return bina1_t, bina2_t, mm_t, binb_t

            def score_jcs(tiles, rep, b, ns, jcs, last=False):
                bina1_t, bina2_t, mm_t, binb_t = tiles
                hT = bina1_t[:, OFF_HT:OFF_HT + N]
                for jc in jcs:
                    e = [pse.tile([128, N], f32, tag="e", name=f"e{rep}_{b}_{jc}_{k}")
                         for k in range(4)]
                    for k in range(4):
                        # e_k[j, i] = sum_d hT[d, j-chunk] * (a_k . h)^T[d, i]
                        nc.tensor.matmul(
                            e[k][:],
                            hT[:, jc * 128:(jc + 1) * 128],
                            (bina1_t[:, (1 + k) * N:(2 + k) * N] if k < 2
                             else bina2_t[:, (k - 2) * N:(k - 1) * N]),
                            start=True,
                            stop=True,
                        )

                    b0m = mm_t[:, jc * N:(jc + 1) * N]
                    b1m = mm_t[:, (JC + jc) * N:(JC + jc + 1) * N]

                    # 4-way select ladder -> e[0] holds e_{adj-1}
                    nc.vector.copy_predicated(e[0][:], b0m, e[1][:])
                    nc.vector.copy_predicated(e[2][:], b0m, e[3][:])
                    nc.vector.copy_predicated(e[0][:], b1m, e[2][:])

                    npre = work.tile([128, N], bf16)
                    nexp = work.tile([128, N], bf16)
                    n = npool.tile([128, N], bf16, tag="n", name=f"n{rep}_{b}_{jc}")
                    if last and jc == JC - 1:
                        # drain tail: chunk the final funnel in halves (the
                        # aggregation consumes n in 128-wide slices) with the
                        # kill on the idle DVE, so the tail aggs start earlier
                        for hf in range(2):
                            sl = slice(hf * 256, (hf + 1) * 256)
                            nc.scalar.activation(
                                npre[:, sl], e[0][:, sl], act.Prelu,
                                alpha=LEAKY_ALPHA
                            )
                            nc.scalar.activation(nexp[:, sl], npre[:, sl], act.Exp)
                            nc.vector.tensor_mul(
                                n[:, sl],
                                binb_t[:, OFF_MZ + jc * N + hf * 256:
                                       OFF_MZ + jc * N + (hf + 1) * 256],
                                nexp[:, sl],
                            )
                    else:
                        nc.scalar.activation(
                            npre[:], e[0][:], act.Prelu, alpha=LEAKY_ALPHA
                        )
                        nc.scalar.activation(nexp[:], npre[:], act.Exp)
                        # n = nexp * mz  (adj==0 -> n == 0), on Pool, off the
                        # ACT critical chain
                        nc.gpsimd.tensor_mul(
                            n[:],
                            binb_t[:, OFF_MZ + jc * N:OFF_MZ + (jc + 1) * N],
                            nexp[:],
                        )
                    ns.append(n)

            def agg_pair(st, icp, last=False):

                rep, b, ns, binb_t, outt = st
                # two aggs packed in one PSUM bank (528B each); the last
                # batch puts its second pair in a (drained) score bank so
                # both pairs accumulate concurrently
                agg = pse.tile([128, N], f32, tag="e",
                               name=f"agg{rep}_{b}_{icp}")[:, 0:2 * HAUG] \
                    .rearrange("p (two h) -> p two h", two=2)
                for half in range(2):
                    ic = 2 * icp + half
                    for jc in range(JC):
                        # out_un[i-chunk, 0:129] += n[:, i-chunk].T @ [h | 1]
                        nc.tensor.matmul(
                            agg[:, half, 0:DIM + 1],
                            ns[jc][:, ic * 128:(ic + 1) * 128],
                            binb_t[:, OFF_HAUG + jc * HAUG:OFF_HAUG + jc * HAUG + DIM + 1],
                            start=(jc == 0),
                            stop=(jc == JC - 1),
                        )
                # ship agg + Z column unnormalized; host divides by Z.
                # The last batch's second pair copies on the (idle) DVE so
                # both copies run concurrently in the drain tail.
                if last and icp == 1:
                    nc.vector.tensor_copy(
                        outt[:, 2 * icp:2 * icp + 2, 0:DIM + 1],
                        agg[:, :, 0:DIM + 1],
                    )
                else:
                    nc.scalar.activation(
                        outt[:, 2 * icp:2 * icp + 2, 0:DIM + 1],
                        agg[:, :, 0:DIM + 1], act.Copy
                    )
                nc.sync.dma_start(
                    out=out_d[b, :, 2 * icp:2 * icp + 2],
                    in_=outt[:, 2 * icp:2 * icp + 2],
                )

            # software pipeline: batch b-1's two aggregation pairs are
            # emitted between batch b's jc-pairs, so agg pair 1 (which waits
            # on pair 0's ACT copy) never blocks batch b+1's score matmuls
            # on the in-order PE queue
            pend = None
            seq = [(r, bb) for r in range(repeat) for bb in range(BPC)]
            for i, (rep, b) in enumerate(seq):
                last = i == len(seq) - 1
                tiles = score_dma(rep, b, first=(i == 0))
                outt = outp.tile([128, IC, DIM + 1], f16, tag="outt",
                                 name=f"outt{rep}_{b}")
                ns = []
                score_jcs(tiles, rep, b, ns, [0, 1, 2], last=last)
                if pend is not None:
                    agg_pair(pend, 0)
                score_jcs(tiles, rep, b, ns, [3], last=last)
                if pend is not None:
                    agg_pair(pend, 1)
                pend = (rep, b, ns, tiles[3], outt)
            agg_pair(pend, 0, last=True)
            agg_pair(pend, 1, last=True)

    nc.compile()
    return nc


def _get_nc():
    if "nc" not in _CACHE:
        _CACHE["nc"] = _build_nc()
    return _CACHE["nc"]


def _host_prep(hidden, adj, input_times, a0, a1, a2, a3, p0, p1):
    import ml_dtypes

    bf16 = ml_dtypes.bfloat16

    hidden = np.asarray(hidden, dtype=np.float32)
    adj = np.asarray(adj)
    input_times = np.asarray(input_times, dtype=np.float32)

    te = np.asarray(p0, np.float32) * np.exp(-input_times) + np.asarray(p1, np.float32)
    h = np.concatenate([hidden[:, :, :-1], te[:, :, None]], axis=2)      # [B,N,128] f32

    hT = np.swapaxes(h, 1, 2)                                            # [B,128,N]
    A = np.stack([a0, a1, a2, a3], 0).astype(np.float32)                 # [4,128]

    bina = np.zeros((B, 128, BINAW), bf16)
    bina[:, :, OFF_HT:OFF_HT + N] = hT.astype(bf16)
    for k in range(4):
        bina[:, :, OFF_HTK + k * N:OFF_HTK + (k + 1) * N] = \
            (A[k][None, :, None] * hT).astype(bf16)

    # haug[b, jp, jc, c] = h[b, jc*128+jp, c] (+ ones col)
    binb = np.zeros((B, 128, BINBW), bf16)
    haug = np.zeros((B, N, HAUG), np.float32)
    haug[:, :, :DIM] = h
    haug[:, :, DIM] = 1.0
    haug = haug.reshape(B, JC, 128, HAUG).transpose(0, 2, 1, 3)
    binb[:, :, OFF_HAUG:OFF_HAUG + JC * HAUG] = \
        haug.reshape(B, 128, JC * HAUG).astype(bf16)

    def chunkT(m):
        # mask[b, i, j] -> transposed + chunked [b, jp, jc*N + i]
        mT = np.swapaxes(m, 1, 2)
        return mT.reshape(B, JC, 128, N).transpose(0, 2, 1, 3).reshape(B, 128, JC * N)

    mz = (adj != 0).astype(np.float32)
    binb[:, :, OFF_MZ:] = chunkT(mz).astype(bf16)

    mmb = np.zeros((B, 128, MMW), np.uint8)
    b0 = (((adj - 1) & 1) * (adj > 0)).astype(np.uint8)
    b1 = (adj >= 3).astype(np.uint8)
    mmb[:, :, :JC * N] = chunkT(b0)
    mmb[:, :, JC * N:] = chunkT(b1)

    in_maps = []
    for c in range(NCORES):
        s = slice(c * BPC, (c + 1) * BPC)
        in_maps.append({"bina": bina[s], "binb": binb[s], "mm": mmb[s]})
    return in_maps


def run(inputs, trace=False, **spmd_kwargs):
    """Full pipeline; returns (output, BassKernelResults)."""
    from concourse import bass_utils

    in_maps = _host_prep(**inputs)
    nc = _get_nc()
    res = bass_utils.run_bass_kernel_spmd(
        nc, in_maps, core_ids=list(range(NCORES)), trace=trace, **spmd_kwargs
    )
    outs = []
    for r in res.results:
        o = np.asarray(r["out"], np.float32)          # [BPC, 128, IC, DIM+1]
        o = o[:, :, :, 0:DIM] / o[:, :, :, DIM:DIM + 1]
        outs.append(o.transpose(0, 2, 1, 3).reshape(BPC, N, DIM))
    full = np.concatenate(outs, axis=0)
    return full, res


def kernel(**inputs) -> np.ndarray:
    out, _ = run(inputs, trace=False)
    return out
